# revision 40
# baseline (speedup 1.0000x reference)
"""Trainium2 Bass kernel for autoregressive MADE Gaussian sampling.

B=4096, D=64, C=128, H=512.  Data-parallel over 8 NeuronCores (512 batch
rows each).  Inside each core the 64-step autoregressive scan runs as an
incremental computation with 2 independent batch sub-chains software-
pipelined half a step apart.

Design notes (restructure over the tuned baseline):
  - zs row layout is trivial: row k (0..63) = sp_k*eps_k, row 64+k =
    mu_k.
  - l1acc is RESIDENT per tile: four [128,nb] PSUM accumulators per
    chain live the whole kernel.  Context matmuls run once at startup
    (hidden under the DMA wait); tile entries keep only the catchup
    matmul + frozen-h2partial work - no memset / context on the path.
  - selection matmuls are 3-way (one sel per THREE degrees, slots at
    PSUM bases 0/32/64 - matmul output bases past 64 are unsupported).
  - QPK (mean->l1 injection) runs per step: relu(d+1) needs group d's
    mean contribution, so it cannot be batched across steps.
  - layer-1 sp contributions: per step one K=32 matmul from the zs sp
    quarter (W1SPE row k = sp row of z_k).
  - PSUM start=True matmuls mark their whole 2KB bank row pending-zero,
    so TP (sel/active accumulation) gets a bank per chain; OUT/l1acc
    banks only ever see start=False onto memset content.
  - Emission is software-pipelined: chain 1's late ops (l3, qpk, exp,
    ln, mult) are emitted at the head of the NEXT step.
  - z-update: softplus as exp+ln(1+x) on ACT (native softplus table is
    absent on this HW); relu / h2g-relu / eps-mult on DVE.
"""

import os

import numpy as np
from ml_dtypes import bfloat16

import concourse.bass as bass
import concourse.bacc as bacc
import concourse.mybir as mybir
from concourse import tile
from concourse.bass_utils import run_bass_kernel_spmd

B, D, C, H = 4096, 64, 128, 512
NCORES = 8
BL = B // NCORES          # 512 batch rows per core
NCHAIN = 2
NBS = [256, 256]
COFF = [0, 256]
F32 = mybir.dt.float32
BF16 = mybir.dt.bfloat16
AF = mybir.ActivationFunctionType
ALU = mybir.AluOpType

GMAX = 9                  # max units per degree group (ceil(512/63))

# Softplus is absent from this HW's activation-table config (gen3
# act_info.json has no softplus entry -> device fault), so softplus runs
# as exp then ln(1+x) on the scalar engine.
USE_NATIVE_SOFTPLUS = os.environ.get("KSOFTPLUS", "0") == "1"


def _zrow(k):
    """zs layout: row k = sp_k*eps_k, row 64+k = mu_k."""
    return 64 + k, k


def _degree_structure():
    m_h = (np.arange(H) % (D - 1)) + 1          # hidden degrees 1..63
    perm = np.argsort(m_h, kind="stable")
    deg = m_h[perm]
    off = np.zeros(D, np.int64)
    cnt = np.zeros(D, np.int64)
    for d in range(1, D):
        idx = np.nonzero(deg == d)[0]
        off[d], cnt[d] = idx[0], len(idx)
    return perm, off, cnt


def _pack_host(W1, b1, W2, b2, W3, b3):
    """Mask, permute and pack the MADE weights into on-chip layouts."""
    perm, off, cnt = _degree_structure()
    m_in = np.arange(1, D + 1)
    m_h = (np.arange(H) % (D - 1)) + 1
    M1 = np.concatenate([m_h[None, :] >= m_in[:, None], np.ones((C, H), bool)], 0)
    M2 = m_h[None, :] >= m_h[:, None]
    m_out = np.tile(np.arange(1, D + 1), 2)
    M3 = m_out[None, :] > m_h[:, None]

    W1m = (W1 * M1).astype(np.float32)
    W1zp = W1m[:D][:, perm]                      # (64, 512) z-row weights
    W1c = np.ascontiguousarray(W1m[D:][:, perm]) # (128, 512) context weights
    W2p = ((W2 * M2)[perm][:, perm]).astype(np.float32)   # (512, 512)
    W2pk = np.concatenate([W2p[kt * 128:(kt + 1) * 128] for kt in range(4)], 1)
    W3p = ((W3 * M3)[perm]).astype(np.float32)   # (512, 128)

    tile_of = (off // 128).astype(np.int64)      # tile index per degree
    tile_of[0] = 0
    d0 = {}
    for d in range(1, D):
        t = int(tile_of[d])
        if t not in d0:
            d0[t] = d

    # W1SPE: per-degree K=32 weights adding the sp*eps row of z_{d-1}.
    # Row (32q + r) = k matches the zs sp row; col block r holds the
    # CURRENT tile's 128 h1 units.
    W1SPE = np.zeros((64, 32 * 128), np.float32)
    for d in range(1, D):
        k = d - 1
        r = k % 32
        t = int(tile_of[d])
        W1SPE[k, r * 128:(r + 1) * 128] = W1zp[k, t * 128:(t + 1) * 128]

    # QPK base blocks: mean contributions to layer-1 via h2g (exact).
    QPKb = np.zeros((GMAX, 63 * 128), np.float32)
    for d in range(1, D):
        g0, n = int(off[d]), int(cnt[d])
        t = int(tile_of[d])
        QPKb[:n, (d - 1) * 128:d * 128] = \
            W3p[g0:g0 + n, 0:D] @ W1zp[:, t * 128:(t + 1) * 128]

    # qpk(d) must land between h2g(d) and relu(d+1) - relu(d+1) needs
    # group d's mean contribution - so it cannot be batched across steps.
    qpk_at = {d: d - 1 for d in range(1, D)
              if d + 1 < D and int(tile_of[d + 1]) == int(tile_of[d])}
    QPK2 = QPKb
    NQB = 63

    # W1ZCAT: catchup weights per tile t in {1,2,3}: mu rows cover ALL k
    # (partial means at entry are completed later by the QPK matmuls);
    # sp rows cover k <= d0(t)-2 (the step-d0 W1SPE matmul adds k=d0-1).
    W1ZCAT = np.zeros((128, 3 * 128), np.float32)
    for t in (1, 2, 3):
        j = t - 1
        for k in range(D):
            mu_r, sp_r = _zrow(k)
            w = W1zp[k, t * 128:(t + 1) * 128]
            W1ZCAT[mu_r, j * 128:(j + 1) * 128] = w
            if k <= int(d0[t]) - 2:
                W1ZCAT[sp_r, j * 128:(j + 1) * 128] = w

    # W3GR: group-major layer-3 weights; output columns land on the
    # trivial zs/OUT row layout (sp row k = k, mu row k = 64+k).
    W3GR = np.zeros((GMAX, 63 * 128), np.float32)
    for d in range(1, D):
        g0, n = int(off[d]), int(cnt[d])
        W3GR[:n, (d - 1) * 128:(d - 1) * 128 + 64] = \
            W3p[g0:g0 + n, D:2 * D]
        W3GR[:n, (d - 1) * 128 + 64:d * 128] = W3p[g0:g0 + n, 0:D]

    # SELPK3: 3-way one-hot selection (matmul PSUM output bases are
    # limited to 0/32/64).  Block p covers degrees d0t+3m..d0t+3m+2 of
    # tile t>=1 (the 16th degree gets its own block); slot s at TP
    # partition base 32s.
    triads = []
    slot_of = {}          # degree -> slot 0..2 (t>=1)
    for t in (1, 2, 3):
        dstart = int(d0[t])
        for m in range(6):
            triads.append(tuple(dstart + 3 * m + s for s in range(3)
                                if 3 * m + s < 16))
    SELPK4 = np.zeros((128, len(triads) * 128), np.float32)
    for p, degs in enumerate(triads):
        for s, dd in enumerate(degs):
            g0l, n = int(off[dd]) - 128 * int(tile_of[dd]), int(cnt[dd])
            for m in range(n):
                SELPK4[g0l + m, p * 128 + 32 * s + m] = 1.0
            slot_of[dd] = s

    # IBLK: final assembly z = mu + sp*eps from rows (64+j, j).
    IBLK = np.zeros((128, D), np.float32)
    for j in range(D):
        mu_r, sp_r = _zrow(j)
        IBLK[mu_r, j] = 1.0
        IBLK[sp_r, j] = 1.0

    packed = {
        "w1c": W1c, "w1spe": W1SPE, "qpk2": QPK2, "w1zcat": W1ZCAT,
        "w2pk": np.ascontiguousarray(W2pk), "w3gr": W3GR,
        "selpk4": SELPK4, "iblk": IBLK,
    }
    meta = dict(off=off, cnt=cnt, tile_of=tile_of, d0=d0,
                qpk_at=qpk_at, nqb=NQB, slot_of=slot_of,
                nquads=len(triads))
    return packed, meta


def _patch_act_tables():
    import concourse.hw_specs as hw
    orig = hw.get_activation_tables("gen3")
    if USE_NATIVE_SOFTPLUS:
        ours = {AF.Softplus, AF.Relu, AF.Copy, AF.Identity}
        home = "softplus_and_others"
    else:
        ours = {AF.Exp, AF.Ln, AF.Relu, AF.Copy, AF.Identity}
        home = "natural_log_exp_and_others"
    patched = {}
    for name, fns in orig.items():
        patched[name] = (set(fns) | ours) if name == home else (set(fns) - ours)
    bacc.get_activation_tables = lambda arch: patched


def _build_nc(meta):
    off, cnt, tile_of, d0 = meta["off"], meta["cnt"], meta["tile_of"], meta["d0"]
    qpk_at, nqb, slot_of = meta["qpk_at"], meta["nqb"], meta["slot_of"]
    nquads = meta["nquads"]

    _patch_act_tables()
    nc = bacc.Bacc(None, target_bir_lowering=False)
    dp = {}
    dp["qT"] = nc.declare_dram_parameter("qT", [C, BL], BF16, isOutput=False)
    dp["epsT"] = nc.declare_dram_parameter("epsT", [D, BL], BF16, isOutput=False)
    dp["w1c"] = nc.declare_dram_parameter("w1c", [C, H], BF16, isOutput=False)
    dp["w1spe"] = nc.declare_dram_parameter("w1spe", [64, 32 * 128], BF16, isOutput=False)
    dp["qpk2"] = nc.declare_dram_parameter("qpk2", [GMAX, 63 * 128], BF16, isOutput=False)
    dp["w1zcat"] = nc.declare_dram_parameter("w1zcat", [128, 3 * 128], BF16, isOutput=False)
    dp["w2pk"] = nc.declare_dram_parameter("w2pk", [128, 4 * H], BF16, isOutput=False)
    dp["w3gr"] = nc.declare_dram_parameter("w3gr", [GMAX, 63 * 128], BF16, isOutput=False)
    dp["selpk4"] = nc.declare_dram_parameter("selpk4", [128, nquads * 128], BF16, isOutput=False)
    dp["iblk"] = nc.declare_dram_parameter("iblk", [128, D], BF16, isOutput=False)
    out_dram = nc.declare_dram_parameter("out", [D, BL], F32, isOutput=True)

    KSTEPS = int(os.environ.get("KSTEPS", str(D)))
    KDUM = int(os.environ.get("KDUM", "0"))
    KDBG = os.environ.get("KDBG", "0") == "1"
    dbg = {}
    if KDBG:
        dbg["zs0"] = nc.declare_dram_parameter("dbg_zs0", [128, 256], BF16, isOutput=True)
        dbg["h1sb0"] = nc.declare_dram_parameter("dbg_h1sb0", [128, 1024], BF16, isOutput=True)
        dbg["OUT0"] = nc.declare_dram_parameter("dbg_OUT0", [128, 256], F32, isOutput=True)
        dbg["h2psb0"] = nc.declare_dram_parameter("dbg_h2psb0", [128, 256], BF16, isOutput=True)
        dbg["L1A0"] = nc.declare_dram_parameter("dbg_L1A0", [128, 512], F32, isOutput=True)
        dbg["TP0"] = nc.declare_dram_parameter("dbg_TP0", [128, 256], F32, isOutput=True)

    with tile.TileContext(nc) as tc:
        with (
            tc.tile_pool(name="const", bufs=1) as cpool,
            tc.tile_pool(name="work", bufs=1) as wpool,
            tc.tile_pool(name="h2g", bufs=2) as gpool,
            tc.tile_pool(name="psL", bufs=1, space="PSUM") as psL,
            tc.tile_pool(name="psO", bufs=1, space="PSUM") as psO,
            tc.tile_pool(name="psP", bufs=1, space="PSUM") as psP,
            tc.tile_pool(name="psS", bufs=1, space="PSUM") as psS,
        ):
            qT = cpool.tile([C, BL], BF16, tag="qT")
            epsb = cpool.tile([64, BL], BF16, tag="epsb")
            w1c = cpool.tile([C, H], BF16, tag="w1c")
            w1spe = cpool.tile([64, 32 * 128], BF16, tag="w1spe")
            qpk2 = cpool.tile([GMAX, 63 * 128], BF16, tag="qpk2")
            w1zcat = cpool.tile([128, 3 * 128], BF16, tag="w1zcat")
            w2pk = cpool.tile([128, 4 * H], BF16, tag="w2pk")
            w3gr = cpool.tile([GMAX, 63 * 128], BF16, tag="w3gr")
            selpk4 = cpool.tile([128, nquads * 128], BF16, tag="selpk4")
            iblk = cpool.tile([128, D], BF16, tag="iblk")
            zout = wpool.tile([D, BL], F32, tag="zout")

            # Startup DMAs: first-needed tensors first, spread across
            # three issue queues; later weights stream in behind the
            # first steps.
            nc.scalar.dma_start(w1c[:, 0:128], dp["w1c"][:, 0:128])
            nc.gpsimd.dma_start(qT[:, 0:128], dp["qT"][:, 0:128])
            nc.sync.dma_start(qT[:, 128:256], dp["qT"][:, 128:256])
            nc.gpsimd.dma_start(qT[:, 256:384], dp["qT"][:, 256:384])
            nc.sync.dma_start(qT[:, 384:512], dp["qT"][:, 384:512])
            nc.scalar.dma_start(epsb[:, :], dp["epsT"][:, :])
            nc.gpsimd.dma_start(w1spe[0:32, 0:1024], dp["w1spe"][0:32, 0:1024])
            nc.sync.dma_start(w2pk[:, 0:H], dp["w2pk"][:, 0:H])
            nc.scalar.dma_start(w3gr[:, 0:1024], dp["w3gr"][:, 0:1024])
            nc.gpsimd.dma_start(qpk2[:, 0:1024], dp["qpk2"][:, 0:1024])
            nc.sync.dma_start(w1spe[0:32, 1024:4096],
                              dp["w1spe"][0:32, 1024:4096])
            nc.scalar.dma_start(w1c[:, 128:512], dp["w1c"][:, 128:512])
            nc.gpsimd.dma_start(w3gr[:, 1024:4096], dp["w3gr"][:, 1024:4096])
            nc.sync.dma_start(qpk2[:, 1024:4096], dp["qpk2"][:, 1024:4096])
            for kt in range(1, 4):
                eng = (None, nc.scalar, nc.gpsimd, nc.sync)[kt]
                eng.dma_start(w2pk[:, kt * H:(kt + 1) * H],
                              dp["w2pk"][:, kt * H:(kt + 1) * H])
            nc.scalar.dma_start(w1zcat[:, :], dp["w1zcat"][:, :])
            nc.gpsimd.dma_start(selpk4[:, :], dp["selpk4"][:, :])
            nc.sync.dma_start(w3gr[:, 4096:63 * 128], dp["w3gr"][:, 4096:63 * 128])
            nc.scalar.dma_start(qpk2[:, 4096:63 * 128], dp["qpk2"][:, 4096:63 * 128])
            nc.gpsimd.dma_start(w1spe[32:64, :], dp["w1spe"][32:64, :])
            nc.sync.dma_start(iblk[:, :], dp["iblk"][:, :])

            zs, h1sb, sp2, h2psb = {}, {}, {}, {}
            L1A, L1B, OUT, TP = {}, {}, {}, {}
            # OUT chains share one PSUM bank (only start=False matmuls land
            # there, onto memset-zeroed content).  TP gets a bank PER chain:
            # a start=True matmul marks the whole 2KB bank row pending-zero
            # on its partitions, so interleaving two chains' start=True sel
            # ops in one bank silently wipes the other chain's accumulands.
            spdm = psS.tile([128, BL], F32, tag="spdm", name="spdm")
            sp1 = spdm[:, 0:256]     # exp scratch (ACT-only; pending-zero
            dumm = spdm[:, 256:512]  # marks from dummy matmuls are harmless)
            OUTT = psO.tile([128, BL], F32, tag="OUTT", name="OUTT")
            for ch in range(NCHAIN):
                nb = NBS[ch]
                c0 = COFF[ch]
                zs[ch] = wpool.tile([128, nb], BF16, tag=f"zs{ch}", name=f"zs{ch}")
                h1sb[ch] = wpool.tile([128, 4 * nb], BF16, tag=f"h1sb{ch}", name=f"h1sb{ch}")
                sp2[ch] = wpool.tile([64, nb], BF16, tag=f"sp2{ch}", name=f"sp2{ch}")
                h2psb[ch] = wpool.tile([128, nb], BF16, tag=f"h2psb{ch}", name=f"h2psb{ch}")
                L1A[ch] = psL.tile([128, 2 * nb], F32, tag=f"L1A{ch}", name=f"L1A{ch}")
                L1B[ch] = psL.tile([128, 2 * nb], F32, tag=f"L1B{ch}", name=f"L1B{ch}")
                OUT[ch] = OUTT[:, c0:c0 + nb]
                TP[ch] = psP.tile([128, nb], F32, tag=f"TP{ch}", name=f"TP{ch}")
                nc.gpsimd.memset(h1sb[ch][:, :], 0.0)
                nc.gpsimd.memset(zs[ch][:, :], 0.0)
                nc.vector.memset(L1A[ch][:, :], 0.0)
                nc.vector.memset(L1B[ch][:, :], 0.0)
                nc.vector.memset(OUT[ch][:, :], 0.0)

            def l1v(ch, t):
                nb = NBS[ch]
                tl = L1A[ch] if t < 2 else L1B[ch]
                j = t % 2
                return tl[:, j * nb:(j + 1) * nb]

            # Startup context matmuls: all four resident l1acc tiles get
            # their context contribution while DMAs stream (off-path).
            for ch in range(NCHAIN):
                nb = NBS[ch]
                c0 = COFF[ch]
                for t in range(4):
                    nc.tensor.matmul(l1v(ch, t),
                                     w1c[:, t * 128:(t + 1) * 128],
                                     qT[:, c0:c0 + nb],
                                     start=False, stop=True,
                                     skip_group_check=True)

            h2g_t = {}        # live h2g tile per chain

            def emit_entry(ch, i):
                """Tile-entry work (t>=1): mu refresh, catchup, frozen
                h2partial."""
                nb = NBS[ch]
                t = int(tile_of[i])
                nc.vector.tensor_copy(zs[ch][64:128, :], OUT[ch][64:128, :])
                j = t - 1
                nc.tensor.matmul(l1v(ch, t),
                                 w1zcat[:, j * 128:(j + 1) * 128],
                                 zs[ch][:, :],
                                 start=False, stop=True,
                                 skip_group_check=True)
                ph2p = TP[ch][:, :]
                for kt in range(t):
                    nc.tensor.matmul(
                        ph2p,
                        w2pk[:, kt * H + t * 128:kt * H + (t + 1) * 128],
                        h1sb[ch][:, kt * nb:(kt + 1) * nb],
                        start=(kt == 0), stop=(kt == t - 1))
                nc.scalar.activation(h2psb[ch][:, :], ph2p, AF.Copy)

            def emit_spe_sel(ch, i):
                """spe-matmul + 4-way selection (PE)."""
                if i < 1:
                    return
                k = i - 1
                q, r = divmod(k, 32)
                t = int(tile_of[i])
                nc.tensor.matmul(l1v(ch, t),
                                 w1spe[32 * q:32 * q + 32,
                                       r * 128:(r + 1) * 128],
                                 zs[ch][32 * q:32 * q + 32, :],
                                 start=False, stop=True, skip_group_check=True)
                d = i
                if t >= 1 and slot_of[d] == 0:
                    p = 6 * (t - 1) + (d - int(d0[t])) // 3
                    nc.tensor.matmul(TP[ch][0:73, :],
                                     selpk4[:, p * 128:p * 128 + 73],
                                     h2psb[ch][:, :],
                                     start=True, stop=True)

            def emit_relu(ch, i):
                """l1 relu (DVE)."""
                if i < 1:
                    return
                nb = NBS[ch]
                t = int(tile_of[i])
                nc.vector.tensor_scalar_max(h1sb[ch][:, t * nb:(t + 1) * nb],
                                            l1v(ch, t), 0.0)

            def emit_active_h2g(ch, i):
                """active-tile matmul (PE) + h2g relu (DVE)."""
                if i < 1:
                    return
                nb = NBS[ch]
                d = i
                t = int(tile_of[d])
                g0, n = int(off[d]), int(cnt[d])
                s = slot_of[d] if t >= 1 else 0
                sb = 32 * s
                nc.tensor.matmul(TP[ch][sb:sb + n, :],
                                 w2pk[:, t * H + g0:t * H + g0 + n],
                                 h1sb[ch][:, t * nb:(t + 1) * nb],
                                 start=(t == 0), stop=True,
                                 skip_group_check=(t >= 1))
                h2g = gpool.tile([GMAX, nb], BF16, tag=f"h2g{ch}",
                                 name=f"h2g{ch}_{i}")
                nc.vector.tensor_scalar_max(h2g[0:n, :], TP[ch][sb:sb + n, :],
                                            0.0)
                h2g_t[ch] = h2g

            def emit_late(ch, i, h2g):
                """l3 matmul + qpk matmul (the PE ops that finish step i)."""
                if i < 1:
                    return
                d = i
                n = int(cnt[d])
                nc.tensor.matmul(OUT[ch][:, :],
                                 w3gr[0:n, (d - 1) * 128:d * 128],
                                 h2g[0:n, :], start=False, stop=True,
                                 skip_group_check=True)
                if d in qpk_at:
                    t = int(tile_of[d])
                    nc.tensor.matmul(l1v(ch, t),
                                     qpk2[0:n, (d - 1) * 128:d * 128],
                                     h2g[0:n, :],
                                     start=False, stop=True,
                                     skip_group_check=True)
                # PE DVFS pump: the PE only reaches 2.4 GHz after ~3us of
                # near-continuous busy; these anchored throwaway matmuls
                # (ready only once this step's h2g exists, so the scheduler
                # cannot hoist them) fill PE idle gaps to keep the clock up.
                for _ in range(KDUM):
                    nc.tensor.matmul(dumm[:, :],
                                     w3gr[0:n, (d - 1) * 128:d * 128],
                                     h2g[0:n, :], start=True, stop=True)

            def emit_softplus(ch, i):
                """exp + ln for step i (scalar engine)."""
                nb = NBS[ch]
                c0 = COFF[ch]
                spw = 32 * (i // 32)
                sr = 64 * ch + spw
                if USE_NATIVE_SOFTPLUS:
                    nc.scalar.activation(sp2[ch][spw:spw + 32, :],
                                         OUT[ch][spw:spw + 32, :],
                                         AF.Softplus)
                else:
                    nc.scalar.activation(sp1[sr:sr + 32, 0:nb],
                                         OUT[ch][spw:spw + 32, :], AF.Exp)
                    nc.scalar.activation(sp2[ch][spw:spw + 32, :],
                                         sp1[sr:sr + 32, 0:nb],
                                         AF.Ln, bias=1.0)

            def emit_mult(ch, i):
                """eps-mult for step i (DVE).  (Tried on GPSIMD to dodge the
                ~600ns DVE queue wait: Pool takes 661ns/op + dispatch and
                lost 8us net.)"""
                nb = NBS[ch]
                c0 = COFF[ch]
                spw = 32 * (i // 32)
                nc.vector.tensor_tensor(zs[ch][spw:spw + 32, :],
                                        sp2[ch][spw:spw + 32, :],
                                        epsb[spw:spw + 32, c0:c0 + nb],
                                        ALU.mult)

            # Software-pipelined emission: chain 1 trails half a step.
            h2g_live = {}     # (ch, i) -> h2g tile for deferred late ops
            for i in range(KSTEPS):
                nxt_entry = (i + 1 < KSTEPS
                             and d0.get(int(tile_of[i + 1])) == i + 1
                             and int(tile_of[i + 1]) >= 1)
                if i >= 1:
                    emit_late(1, i - 1, h2g_live.pop((1, i - 1), None))
                    emit_softplus(1, i - 1)
                if (i >= 1 and d0.get(int(tile_of[i])) == i
                        and int(tile_of[i]) >= 1):
                    emit_entry(1, i)
                emit_spe_sel(0, i)
                emit_relu(0, i)
                if i >= 1:
                    emit_mult(1, i - 1)
                emit_active_h2g(0, i)
                emit_spe_sel(1, i)
                emit_relu(1, i)
                h2g_live[(0, i)] = h2g_t.get(0)
                emit_late(0, i, h2g_live.pop((0, i), None))
                emit_softplus(0, i)
                if nxt_entry:
                    # hoist next tile's entry prep behind this step's tail
                    emit_entry(0, i + 1)
                emit_mult(0, i)
                emit_active_h2g(1, i)
                h2g_live[(1, i)] = h2g_t.get(1)

            def emit_final(ch):
                # z = mu + sp*eps directly: one DVE add from the f32 PSUM mu
                # rows + bf16 sp*eps rows (replaces the cold-PE IBLK matmul
                # + two copies; also keeps mu at f32 precision).
                nb = NBS[ch]
                c0 = COFF[ch]
                nc.vector.tensor_tensor(zout[:, c0:c0 + nb],
                                        OUT[ch][64:128, :],
                                        zs[ch][0:64, :], ALU.add)
                eng = nc.sync if ch == 0 else nc.scalar
                eng.dma_start(out_dram[:, c0:c0 + nb], zout[:, c0:c0 + nb])

            if KDBG:
                dOUT = wpool.tile([128, 256], F32, tag="dOUT")
                dL1A = wpool.tile([128, 512], F32, tag="dL1A")
                dTP = wpool.tile([128, 256], F32, tag="dTP")
                nc.scalar.activation(dOUT[:, :], OUT[0][:, :], AF.Copy)
                nc.scalar.activation(dL1A[:, :], L1A[0][:, :], AF.Copy)
                nc.scalar.activation(dTP[:, :], TP[0][:, :], AF.Copy)
                nc.sync.dma_start(dbg["TP0"][:, :], dTP[:, :])
                nc.sync.dma_start(dbg["zs0"][:, :], zs[0][:, :])
                nc.sync.dma_start(dbg["h1sb0"][:, :], h1sb[0][:, :])
                nc.sync.dma_start(dbg["OUT0"][:, :], dOUT[:, :])
                nc.sync.dma_start(dbg["h2psb0"][:, :], h2psb[0][:, :])
                nc.sync.dma_start(dbg["L1A0"][:, :], dL1A[:, :])
            i = KSTEPS - 1
            emit_final(0)
            emit_late(1, i, h2g_live.pop((1, i), None))
            emit_softplus(1, i)
            emit_mult(1, i)
            emit_final(1)
    nc.compile()
    return nc


_CACHE = {}


def kernel(q_z_x_params, eps, W1, b1, W2, b2, W3, b3):
    q = np.ascontiguousarray(q_z_x_params, np.float32)
    eps = np.asarray(eps, np.float32)
    packed, meta = _pack_host(
        np.asarray(W1, np.float32), np.asarray(b1, np.float32),
        np.asarray(W2, np.float32), np.asarray(b2, np.float32),
        np.asarray(W3, np.float32), np.asarray(b3, np.float32))

    if "nc" not in _CACHE:
        _CACHE["nc"] = _build_nc(meta)
    nc = _CACHE["nc"]

    bfpacked = {k: v.astype(bfloat16) for k, v in packed.items()}
    in_maps = []
    for c in range(NCORES):
        sl = slice(c * BL, (c + 1) * BL)
        m = dict(bfpacked)
        m["qT"] = np.ascontiguousarray(q[sl].T).astype(bfloat16)
        m["epsT"] = np.ascontiguousarray(eps[sl].T).astype(bfloat16)
        in_maps.append(m)

    res = run_bass_kernel_spmd(nc, in_maps, core_ids=list(range(NCORES)))
    outs = [np.asarray(res.results[c]["out"]).T for c in range(NCORES)]  # (BL, D)
    return np.concatenate(outs, 0).astype(np.float32)


if __name__ == "__main__":
    dat = np.load("/tmp/ref_inputs.npz")
    out = kernel(**{k: dat[k] for k in dat.files})
    ref = np.load("/tmp/ref_out.npy")
    rel = np.linalg.norm(out - ref) / np.linalg.norm(ref)
    print("Relative error:", rel)


# revision 41
# speedup vs baseline: 1.0100x; 1.0100x over previous
"""Trainium2 Bass kernel for autoregressive MADE Gaussian sampling.

B=4096, D=64, C=128, H=512.  Data-parallel over 8 NeuronCores (512 batch
rows each).  Inside each core the 64-step autoregressive scan runs as an
incremental computation with 2 independent batch sub-chains software-
pipelined half a step apart.

Design notes (restructure over the tuned baseline):
  - zs row layout is trivial: row k (0..63) = sp_k*eps_k, row 64+k =
    mu_k.
  - l1acc is RESIDENT per tile: four [128,nb] PSUM accumulators per
    chain live the whole kernel.  Context matmuls run once at startup
    (hidden under the DMA wait); tile entries keep only the catchup
    matmul + frozen-h2partial work - no memset / context on the path.
  - selection matmuls are 3-way (one sel per THREE degrees, slots at
    PSUM bases 0/32/64 - matmul output bases past 64 are unsupported).
  - QPK (mean->l1 injection) runs per step: relu(d+1) needs group d's
    mean contribution, so it cannot be batched across steps.
  - layer-1 sp contributions: per step one K=32 matmul from the zs sp
    quarter (W1SPE row k = sp row of z_k).
  - PSUM start=True matmuls mark their whole 2KB bank row pending-zero,
    so TP (sel/active accumulation) gets a bank per chain; OUT/l1acc
    banks only ever see start=False onto memset content.
  - Emission is software-pipelined: chain 1's late ops (l3, qpk, exp,
    ln, mult) are emitted at the head of the NEXT step.
  - z-update: softplus as exp+ln(1+x) on ACT (native softplus table is
    absent on this HW); relu / h2g-relu / eps-mult on DVE.
"""

import os

import numpy as np
from ml_dtypes import bfloat16

import concourse.bass as bass
import concourse.bacc as bacc
import concourse.mybir as mybir
from concourse import tile
from concourse.bass_utils import run_bass_kernel_spmd

B, D, C, H = 4096, 64, 128, 512
NCORES = 8
BL = B // NCORES          # 512 batch rows per core
NCHAIN = 2
NBS = [256, 256]
COFF = [0, 256]
F32 = mybir.dt.float32
BF16 = mybir.dt.bfloat16
AF = mybir.ActivationFunctionType
ALU = mybir.AluOpType

GMAX = 9                  # max units per degree group (ceil(512/63))

# Softplus is absent from this HW's activation-table config (gen3
# act_info.json has no softplus entry -> device fault), so softplus runs
# as exp then ln(1+x) on the scalar engine.
USE_NATIVE_SOFTPLUS = os.environ.get("KSOFTPLUS", "0") == "1"


def _zrow(k):
    """zs layout: row k = sp_k*eps_k, row 64+k = mu_k."""
    return 64 + k, k


def _degree_structure():
    m_h = (np.arange(H) % (D - 1)) + 1          # hidden degrees 1..63
    perm = np.argsort(m_h, kind="stable")
    deg = m_h[perm]
    off = np.zeros(D, np.int64)
    cnt = np.zeros(D, np.int64)
    for d in range(1, D):
        idx = np.nonzero(deg == d)[0]
        off[d], cnt[d] = idx[0], len(idx)
    return perm, off, cnt


def _pack_host(W1, b1, W2, b2, W3, b3):
    """Mask, permute and pack the MADE weights into on-chip layouts."""
    perm, off, cnt = _degree_structure()
    m_in = np.arange(1, D + 1)
    m_h = (np.arange(H) % (D - 1)) + 1
    M1 = np.concatenate([m_h[None, :] >= m_in[:, None], np.ones((C, H), bool)], 0)
    M2 = m_h[None, :] >= m_h[:, None]
    m_out = np.tile(np.arange(1, D + 1), 2)
    M3 = m_out[None, :] > m_h[:, None]

    W1m = (W1 * M1).astype(np.float32)
    W1zp = W1m[:D][:, perm]                      # (64, 512) z-row weights
    W1c = np.ascontiguousarray(W1m[D:][:, perm]) # (128, 512) context weights
    W2p = ((W2 * M2)[perm][:, perm]).astype(np.float32)   # (512, 512)
    W2pk = np.concatenate([W2p[kt * 128:(kt + 1) * 128] for kt in range(4)], 1)
    W3p = ((W3 * M3)[perm]).astype(np.float32)   # (512, 128)

    tile_of = (off // 128).astype(np.int64)      # tile index per degree
    tile_of[0] = 0
    d0 = {}
    for d in range(1, D):
        t = int(tile_of[d])
        if t not in d0:
            d0[t] = d

    # W1SPE: per-degree K=32 weights adding the sp*eps row of z_{d-1}.
    # Row (32q + r) = k matches the zs sp row; col block r holds the
    # CURRENT tile's 128 h1 units.
    W1SPE = np.zeros((64, 32 * 128), np.float32)
    for d in range(1, D):
        k = d - 1
        r = k % 32
        t = int(tile_of[d])
        W1SPE[k, r * 128:(r + 1) * 128] = W1zp[k, t * 128:(t + 1) * 128]

    # QPK base blocks: mean contributions to layer-1 via h2g (exact).
    QPKb = np.zeros((GMAX, 63 * 128), np.float32)
    for d in range(1, D):
        g0, n = int(off[d]), int(cnt[d])
        t = int(tile_of[d])
        QPKb[:n, (d - 1) * 128:d * 128] = \
            W3p[g0:g0 + n, 0:D] @ W1zp[:, t * 128:(t + 1) * 128]

    # qpk(d) must land between h2g(d) and relu(d+1) - relu(d+1) needs
    # group d's mean contribution - so it cannot be batched across steps.
    qpk_at = {d: d - 1 for d in range(1, D)
              if d + 1 < D and int(tile_of[d + 1]) == int(tile_of[d])}
    QPK2 = QPKb
    NQB = 63

    # W1ZCAT: catchup weights per tile t in {1,2,3}: mu rows cover ALL k
    # (partial means at entry are completed later by the QPK matmuls);
    # sp rows cover k <= d0(t)-2 (the step-d0 W1SPE matmul adds k=d0-1).
    W1ZCAT = np.zeros((128, 3 * 128), np.float32)
    for t in (1, 2, 3):
        j = t - 1
        for k in range(D):
            mu_r, sp_r = _zrow(k)
            w = W1zp[k, t * 128:(t + 1) * 128]
            W1ZCAT[mu_r, j * 128:(j + 1) * 128] = w
            if k <= int(d0[t]) - 2:
                W1ZCAT[sp_r, j * 128:(j + 1) * 128] = w

    # W3GR: group-major layer-3 weights; output columns land on the
    # trivial zs/OUT row layout (sp row k = k, mu row k = 64+k).
    W3GR = np.zeros((GMAX, 63 * 128), np.float32)
    for d in range(1, D):
        g0, n = int(off[d]), int(cnt[d])
        W3GR[:n, (d - 1) * 128:(d - 1) * 128 + 64] = \
            W3p[g0:g0 + n, D:2 * D]
        W3GR[:n, (d - 1) * 128 + 64:d * 128] = W3p[g0:g0 + n, 0:D]

    # SELPK3: 3-way one-hot selection (matmul PSUM output bases are
    # limited to 0/32/64).  Block p covers degrees d0t+3m..d0t+3m+2 of
    # tile t>=1 (the 16th degree gets its own block); slot s at TP
    # partition base 32s.
    triads = []
    slot_of = {}          # degree -> slot 0..2 (t>=1)
    for t in (1, 2, 3):
        dstart = int(d0[t])
        for m in range(6):
            triads.append(tuple(dstart + 3 * m + s for s in range(3)
                                if 3 * m + s < 16))
    SELPK4 = np.zeros((128, len(triads) * 128), np.float32)
    for p, degs in enumerate(triads):
        for s, dd in enumerate(degs):
            g0l, n = int(off[dd]) - 128 * int(tile_of[dd]), int(cnt[dd])
            for m in range(n):
                SELPK4[g0l + m, p * 128 + 32 * s + m] = 1.0
            slot_of[dd] = s

    # IBLK: final assembly z = mu + sp*eps from rows (64+j, j).
    IBLK = np.zeros((128, D), np.float32)
    for j in range(D):
        mu_r, sp_r = _zrow(j)
        IBLK[mu_r, j] = 1.0
        IBLK[sp_r, j] = 1.0

    packed = {
        "w1c": W1c, "w1spe": W1SPE, "qpk2": QPK2, "w1zcat": W1ZCAT,
        "w2pk": np.ascontiguousarray(W2pk), "w3gr": W3GR,
        "selpk4": SELPK4, "iblk": IBLK,
    }
    meta = dict(off=off, cnt=cnt, tile_of=tile_of, d0=d0,
                qpk_at=qpk_at, nqb=NQB, slot_of=slot_of,
                nquads=len(triads))
    return packed, meta


def _patch_act_tables():
    import concourse.hw_specs as hw
    orig = hw.get_activation_tables("gen3")
    if USE_NATIVE_SOFTPLUS:
        ours = {AF.Softplus, AF.Relu, AF.Copy, AF.Identity}
        home = "softplus_and_others"
    else:
        ours = {AF.Exp, AF.Ln, AF.Relu, AF.Copy, AF.Identity}
        home = "natural_log_exp_and_others"
    patched = {}
    for name, fns in orig.items():
        patched[name] = (set(fns) | ours) if name == home else (set(fns) - ours)
    bacc.get_activation_tables = lambda arch: patched


def _build_nc(meta):
    off, cnt, tile_of, d0 = meta["off"], meta["cnt"], meta["tile_of"], meta["d0"]
    qpk_at, nqb, slot_of = meta["qpk_at"], meta["nqb"], meta["slot_of"]
    nquads = meta["nquads"]

    _patch_act_tables()
    nc = bacc.Bacc(None, target_bir_lowering=False)
    dp = {}
    dp["qT"] = nc.declare_dram_parameter("qT", [C, BL], BF16, isOutput=False)
    dp["epsT"] = nc.declare_dram_parameter("epsT", [D, BL], BF16, isOutput=False)
    dp["w1c"] = nc.declare_dram_parameter("w1c", [C, H], BF16, isOutput=False)
    dp["w1spe"] = nc.declare_dram_parameter("w1spe", [64, 32 * 128], BF16, isOutput=False)
    dp["qpk2"] = nc.declare_dram_parameter("qpk2", [GMAX, 63 * 128], BF16, isOutput=False)
    dp["w1zcat"] = nc.declare_dram_parameter("w1zcat", [128, 3 * 128], BF16, isOutput=False)
    dp["w2pk"] = nc.declare_dram_parameter("w2pk", [128, 4 * H], BF16, isOutput=False)
    dp["w3gr"] = nc.declare_dram_parameter("w3gr", [GMAX, 63 * 128], BF16, isOutput=False)
    dp["selpk4"] = nc.declare_dram_parameter("selpk4", [128, nquads * 128], BF16, isOutput=False)
    dp["iblk"] = nc.declare_dram_parameter("iblk", [128, D], BF16, isOutput=False)
    out_dram = nc.declare_dram_parameter("out", [D, BL], F32, isOutput=True)

    KSTEPS = int(os.environ.get("KSTEPS", str(D)))
    KDUM = int(os.environ.get("KDUM", "0"))
    KDBG = os.environ.get("KDBG", "0") == "1"
    dbg = {}
    if KDBG:
        dbg["zs0"] = nc.declare_dram_parameter("dbg_zs0", [128, 256], BF16, isOutput=True)
        dbg["h1sb0"] = nc.declare_dram_parameter("dbg_h1sb0", [128, 1024], BF16, isOutput=True)
        dbg["OUT0"] = nc.declare_dram_parameter("dbg_OUT0", [128, 256], F32, isOutput=True)
        dbg["h2psb0"] = nc.declare_dram_parameter("dbg_h2psb0", [128, 256], BF16, isOutput=True)
        dbg["L1A0"] = nc.declare_dram_parameter("dbg_L1A0", [128, 512], F32, isOutput=True)
        dbg["TP0"] = nc.declare_dram_parameter("dbg_TP0", [128, 256], F32, isOutput=True)

    with tile.TileContext(nc) as tc:
        with (
            tc.tile_pool(name="const", bufs=1) as cpool,
            tc.tile_pool(name="work", bufs=1) as wpool,
            tc.tile_pool(name="h2g", bufs=2) as gpool,
            tc.tile_pool(name="psL", bufs=1, space="PSUM") as psL,
            tc.tile_pool(name="psO", bufs=1, space="PSUM") as psO,
            tc.tile_pool(name="psP", bufs=1, space="PSUM") as psP,
            tc.tile_pool(name="psS", bufs=1, space="PSUM") as psS,
        ):
            qT = cpool.tile([C, BL], BF16, tag="qT")
            epsb = cpool.tile([64, BL], BF16, tag="epsb")
            w1c = cpool.tile([C, H], BF16, tag="w1c")
            w1spe = cpool.tile([64, 32 * 128], BF16, tag="w1spe")
            qpk2 = cpool.tile([GMAX, 63 * 128], BF16, tag="qpk2")
            w1zcat = cpool.tile([128, 3 * 128], BF16, tag="w1zcat")
            w2pk = cpool.tile([128, 4 * H], BF16, tag="w2pk")
            w3gr = cpool.tile([GMAX, 63 * 128], BF16, tag="w3gr")
            selpk4 = cpool.tile([128, nquads * 128], BF16, tag="selpk4")
            iblk = cpool.tile([128, D], BF16, tag="iblk")
            zout = wpool.tile([D, BL], F32, tag="zout")

            # Startup DMAs: first-needed tensors first, spread across
            # three issue queues; later weights stream in behind the
            # first steps.
            nc.scalar.dma_start(w1c[:, 0:128], dp["w1c"][:, 0:128])
            nc.gpsimd.dma_start(qT[:, 0:256], dp["qT"][:, 0:256])
            nc.sync.dma_start(qT[:, 256:512], dp["qT"][:, 256:512])
            nc.scalar.dma_start(epsb[:, :], dp["epsT"][:, :])
            nc.gpsimd.dma_start(w1spe[0:32, 0:1024], dp["w1spe"][0:32, 0:1024])
            nc.sync.dma_start(w2pk[:, 0:H], dp["w2pk"][:, 0:H])
            nc.scalar.dma_start(w3gr[:, 0:1024], dp["w3gr"][:, 0:1024])
            nc.gpsimd.dma_start(qpk2[:, 0:1024], dp["qpk2"][:, 0:1024])
            nc.sync.dma_start(w1spe[0:32, 1024:4096],
                              dp["w1spe"][0:32, 1024:4096])
            nc.scalar.dma_start(w1c[:, 128:512], dp["w1c"][:, 128:512])
            nc.gpsimd.dma_start(w3gr[:, 1024:4096], dp["w3gr"][:, 1024:4096])
            nc.sync.dma_start(qpk2[:, 1024:4096], dp["qpk2"][:, 1024:4096])
            for kt in range(1, 4):
                eng = (None, nc.scalar, nc.gpsimd, nc.sync)[kt]
                eng.dma_start(w2pk[:, kt * H:(kt + 1) * H],
                              dp["w2pk"][:, kt * H:(kt + 1) * H])
            nc.scalar.dma_start(w1zcat[:, :], dp["w1zcat"][:, :])
            nc.gpsimd.dma_start(selpk4[:, :], dp["selpk4"][:, :])
            nc.sync.dma_start(w3gr[:, 4096:63 * 128], dp["w3gr"][:, 4096:63 * 128])
            nc.scalar.dma_start(qpk2[:, 4096:63 * 128], dp["qpk2"][:, 4096:63 * 128])
            nc.gpsimd.dma_start(w1spe[32:64, :], dp["w1spe"][32:64, :])
            nc.sync.dma_start(iblk[:, :], dp["iblk"][:, :])

            zs, h1sb, sp2, h2psb = {}, {}, {}, {}
            L1A, L1B, OUT, TP = {}, {}, {}, {}
            # OUT chains share one PSUM bank (only start=False matmuls land
            # there, onto memset-zeroed content).  TP gets a bank PER chain:
            # a start=True matmul marks the whole 2KB bank row pending-zero
            # on its partitions, so interleaving two chains' start=True sel
            # ops in one bank silently wipes the other chain's accumulands.
            spdm = psS.tile([128, BL], F32, tag="spdm", name="spdm")
            sp1 = spdm[:, 0:256]     # exp scratch (ACT-only; pending-zero
            dumm = spdm[:, 256:512]  # marks from dummy matmuls are harmless)
            OUTT = psO.tile([128, BL], F32, tag="OUTT", name="OUTT")
            for ch in range(NCHAIN):
                nb = NBS[ch]
                c0 = COFF[ch]
                zs[ch] = wpool.tile([128, nb], BF16, tag=f"zs{ch}", name=f"zs{ch}")
                h1sb[ch] = wpool.tile([128, 4 * nb], BF16, tag=f"h1sb{ch}", name=f"h1sb{ch}")
                sp2[ch] = wpool.tile([64, nb], BF16, tag=f"sp2{ch}", name=f"sp2{ch}")
                h2psb[ch] = wpool.tile([128, nb], BF16, tag=f"h2psb{ch}", name=f"h2psb{ch}")
                L1A[ch] = psL.tile([128, 2 * nb], F32, tag=f"L1A{ch}", name=f"L1A{ch}")
                L1B[ch] = psL.tile([128, 2 * nb], F32, tag=f"L1B{ch}", name=f"L1B{ch}")
                OUT[ch] = OUTT[:, c0:c0 + nb]
                TP[ch] = psP.tile([128, nb], F32, tag=f"TP{ch}", name=f"TP{ch}")
                nc.gpsimd.memset(h1sb[ch][:, :], 0.0)
                nc.gpsimd.memset(zs[ch][:, :], 0.0)
                nc.vector.memset(L1A[ch][:, :], 0.0)
                nc.vector.memset(L1B[ch][:, :], 0.0)
                nc.vector.memset(OUT[ch][:, :], 0.0)

            def l1v(ch, t):
                nb = NBS[ch]
                tl = L1A[ch] if t < 2 else L1B[ch]
                j = t % 2
                return tl[:, j * nb:(j + 1) * nb]

            # Startup context matmuls: all four resident l1acc tiles get
            # their context contribution while DMAs stream (off-path).
            for ch in range(NCHAIN):
                nb = NBS[ch]
                c0 = COFF[ch]
                for t in range(4):
                    nc.tensor.matmul(l1v(ch, t),
                                     w1c[:, t * 128:(t + 1) * 128],
                                     qT[:, c0:c0 + nb],
                                     start=False, stop=True,
                                     skip_group_check=True)

            h2g_t = {}        # live h2g tile per chain

            def emit_entry(ch, i):
                """Tile-entry work (t>=1): mu refresh, catchup, frozen
                h2partial."""
                nb = NBS[ch]
                t = int(tile_of[i])
                nc.vector.tensor_copy(zs[ch][64:128, :], OUT[ch][64:128, :])
                j = t - 1
                nc.tensor.matmul(l1v(ch, t),
                                 w1zcat[:, j * 128:(j + 1) * 128],
                                 zs[ch][:, :],
                                 start=False, stop=True,
                                 skip_group_check=True)
                ph2p = TP[ch][:, :]
                for kt in range(t):
                    nc.tensor.matmul(
                        ph2p,
                        w2pk[:, kt * H + t * 128:kt * H + (t + 1) * 128],
                        h1sb[ch][:, kt * nb:(kt + 1) * nb],
                        start=(kt == 0), stop=(kt == t - 1))
                nc.scalar.activation(h2psb[ch][:, :], ph2p, AF.Copy)

            def emit_spe_sel(ch, i):
                """spe-matmul + 4-way selection (PE)."""
                if i < 1:
                    return
                k = i - 1
                q, r = divmod(k, 32)
                t = int(tile_of[i])
                nc.tensor.matmul(l1v(ch, t),
                                 w1spe[32 * q:32 * q + 32,
                                       r * 128:(r + 1) * 128],
                                 zs[ch][32 * q:32 * q + 32, :],
                                 start=False, stop=True, skip_group_check=True)
                d = i
                if t >= 1 and slot_of[d] == 0:
                    p = 6 * (t - 1) + (d - int(d0[t])) // 3
                    nc.tensor.matmul(TP[ch][0:73, :],
                                     selpk4[:, p * 128:p * 128 + 73],
                                     h2psb[ch][:, :],
                                     start=True, stop=True)

            def emit_relu(ch, i):
                """l1 relu (DVE)."""
                if i < 1:
                    return
                nb = NBS[ch]
                t = int(tile_of[i])
                nc.vector.tensor_scalar_max(h1sb[ch][:, t * nb:(t + 1) * nb],
                                            l1v(ch, t), 0.0)

            def emit_active_h2g(ch, i):
                """active-tile matmul (PE) + h2g relu (DVE)."""
                if i < 1:
                    return
                nb = NBS[ch]
                d = i
                t = int(tile_of[d])
                g0, n = int(off[d]), int(cnt[d])
                s = slot_of[d] if t >= 1 else 0
                sb = 32 * s
                nc.tensor.matmul(TP[ch][sb:sb + n, :],
                                 w2pk[:, t * H + g0:t * H + g0 + n],
                                 h1sb[ch][:, t * nb:(t + 1) * nb],
                                 start=(t == 0), stop=True,
                                 skip_group_check=(t >= 1))
                h2g = gpool.tile([GMAX, nb], BF16, tag=f"h2g{ch}",
                                 name=f"h2g{ch}_{i}")
                nc.vector.tensor_scalar_max(h2g[0:n, :], TP[ch][sb:sb + n, :],
                                            0.0)
                h2g_t[ch] = h2g

            def emit_late(ch, i, h2g):
                """l3 matmul + qpk matmul (the PE ops that finish step i)."""
                if i < 1:
                    return
                d = i
                n = int(cnt[d])
                nc.tensor.matmul(OUT[ch][:, :],
                                 w3gr[0:n, (d - 1) * 128:d * 128],
                                 h2g[0:n, :], start=False, stop=True,
                                 skip_group_check=True)
                if d in qpk_at:
                    t = int(tile_of[d])
                    nc.tensor.matmul(l1v(ch, t),
                                     qpk2[0:n, (d - 1) * 128:d * 128],
                                     h2g[0:n, :],
                                     start=False, stop=True,
                                     skip_group_check=True)
                # PE DVFS pump: the PE only reaches 2.4 GHz after ~3us of
                # near-continuous busy; these anchored throwaway matmuls
                # (ready only once this step's h2g exists, so the scheduler
                # cannot hoist them) fill PE idle gaps to keep the clock up.
                for _ in range(KDUM):
                    nc.tensor.matmul(dumm[:, :],
                                     w3gr[0:n, (d - 1) * 128:d * 128],
                                     h2g[0:n, :], start=True, stop=True)

            def emit_softplus(ch, i):
                """exp + ln for step i (scalar engine)."""
                nb = NBS[ch]
                c0 = COFF[ch]
                spw = 32 * (i // 32)
                sr = 64 * ch + spw
                if USE_NATIVE_SOFTPLUS:
                    nc.scalar.activation(sp2[ch][spw:spw + 32, :],
                                         OUT[ch][spw:spw + 32, :],
                                         AF.Softplus)
                else:
                    nc.scalar.activation(sp1[sr:sr + 32, 0:nb],
                                         OUT[ch][spw:spw + 32, :], AF.Exp)
                    nc.scalar.activation(sp2[ch][spw:spw + 32, :],
                                         sp1[sr:sr + 32, 0:nb],
                                         AF.Ln, bias=1.0)

            def emit_mult(ch, i):
                """eps-mult for step i (DVE).  (Tried on GPSIMD to dodge the
                ~600ns DVE queue wait: Pool takes 661ns/op + dispatch and
                lost 8us net.)"""
                nb = NBS[ch]
                c0 = COFF[ch]
                spw = 32 * (i // 32)
                nc.vector.tensor_tensor(zs[ch][spw:spw + 32, :],
                                        sp2[ch][spw:spw + 32, :],
                                        epsb[spw:spw + 32, c0:c0 + nb],
                                        ALU.mult)

            # Software-pipelined emission: chain 1 trails half a step.
            h2g_live = {}     # (ch, i) -> h2g tile for deferred late ops
            for i in range(KSTEPS):
                nxt_entry = (i + 1 < KSTEPS
                             and d0.get(int(tile_of[i + 1])) == i + 1
                             and int(tile_of[i + 1]) >= 1)
                if i >= 1:
                    emit_late(1, i - 1, h2g_live.pop((1, i - 1), None))
                    emit_softplus(1, i - 1)
                if (i >= 1 and d0.get(int(tile_of[i])) == i
                        and int(tile_of[i]) >= 1):
                    emit_entry(1, i)
                emit_spe_sel(0, i)
                emit_relu(0, i)
                if i >= 1:
                    emit_mult(1, i - 1)
                emit_active_h2g(0, i)
                emit_spe_sel(1, i)
                emit_relu(1, i)
                h2g_live[(0, i)] = h2g_t.get(0)
                emit_late(0, i, h2g_live.pop((0, i), None))
                emit_softplus(0, i)
                if nxt_entry:
                    # hoist next tile's entry prep behind this step's tail
                    emit_entry(0, i + 1)
                emit_mult(0, i)
                emit_active_h2g(1, i)
                h2g_live[(1, i)] = h2g_t.get(1)

            def emit_final(ch):
                # z = mu + sp*eps directly: one DVE add from the f32 PSUM mu
                # rows + bf16 sp*eps rows (replaces the cold-PE IBLK matmul
                # + two copies; also keeps mu at f32 precision).
                nb = NBS[ch]
                c0 = COFF[ch]
                nc.vector.tensor_tensor(zout[:, c0:c0 + nb],
                                        OUT[ch][64:128, :],
                                        zs[ch][0:64, :], ALU.add)
                eng = nc.sync if ch == 0 else nc.scalar
                eng.dma_start(out_dram[:, c0:c0 + nb], zout[:, c0:c0 + nb])

            if KDBG:
                dOUT = wpool.tile([128, 256], F32, tag="dOUT")
                dL1A = wpool.tile([128, 512], F32, tag="dL1A")
                dTP = wpool.tile([128, 256], F32, tag="dTP")
                nc.scalar.activation(dOUT[:, :], OUT[0][:, :], AF.Copy)
                nc.scalar.activation(dL1A[:, :], L1A[0][:, :], AF.Copy)
                nc.scalar.activation(dTP[:, :], TP[0][:, :], AF.Copy)
                nc.sync.dma_start(dbg["TP0"][:, :], dTP[:, :])
                nc.sync.dma_start(dbg["zs0"][:, :], zs[0][:, :])
                nc.sync.dma_start(dbg["h1sb0"][:, :], h1sb[0][:, :])
                nc.sync.dma_start(dbg["OUT0"][:, :], dOUT[:, :])
                nc.sync.dma_start(dbg["h2psb0"][:, :], h2psb[0][:, :])
                nc.sync.dma_start(dbg["L1A0"][:, :], dL1A[:, :])
            i = KSTEPS - 1
            emit_final(0)
            emit_late(1, i, h2g_live.pop((1, i), None))
            emit_softplus(1, i)
            emit_mult(1, i)
            emit_final(1)
    nc.compile()
    return nc


_CACHE = {}


def kernel(q_z_x_params, eps, W1, b1, W2, b2, W3, b3):
    q = np.ascontiguousarray(q_z_x_params, np.float32)
    eps = np.asarray(eps, np.float32)
    packed, meta = _pack_host(
        np.asarray(W1, np.float32), np.asarray(b1, np.float32),
        np.asarray(W2, np.float32), np.asarray(b2, np.float32),
        np.asarray(W3, np.float32), np.asarray(b3, np.float32))

    if "nc" not in _CACHE:
        _CACHE["nc"] = _build_nc(meta)
    nc = _CACHE["nc"]

    bfpacked = {k: v.astype(bfloat16) for k, v in packed.items()}
    in_maps = []
    for c in range(NCORES):
        sl = slice(c * BL, (c + 1) * BL)
        m = dict(bfpacked)
        m["qT"] = np.ascontiguousarray(q[sl].T).astype(bfloat16)
        m["epsT"] = np.ascontiguousarray(eps[sl].T).astype(bfloat16)
        in_maps.append(m)

    res = run_bass_kernel_spmd(nc, in_maps, core_ids=list(range(NCORES)))
    outs = [np.asarray(res.results[c]["out"]).T for c in range(NCORES)]  # (BL, D)
    return np.concatenate(outs, 0).astype(np.float32)


if __name__ == "__main__":
    dat = np.load("/tmp/ref_inputs.npz")
    out = kernel(**{k: dat[k] for k in dat.files})
    ref = np.load("/tmp/ref_out.npy")
    rel = np.linalg.norm(out - ref) / np.linalg.norm(ref)
    print("Relative error:", rel)
